# revision 14
# baseline (speedup 1.0000x reference)
"""Trainium2 Bass kernel for gated multi-head attention (AlphaFold-style).

Reference computation (per batch b):
  q = Q @ qw * dk^-0.5; k = K @ kw; v = V @ vw           (per-head projections)
  logits = q @ k^T + bias; W = softmax(logits)
  W = where(mask, W, 0)                                   (post-softmax mask)
  av = W @ v; gate = sigmoid(Q @ gw + g_bias); av *= gate
  out = av @ o_w + o_bias

Sharding: 8 cores; core i handles batch b=i//4 and 4 heads h0=4*(i%4).
Each core returns a partial [LQ, D_MODEL] output (its heads' o-projection
contribution, bf16); host sums the partials per batch and adds o_bias.

v2 design (all-bf16 compute, PE-lean):
  - Host pre-transposes Q,K,V to [A, L] bf16 -> no on-device input
    transposes; projections read XT slabs directly as lhsT/rhs.
  - Head pairs stacked on partitions (base 0/64); QK matmuls for the two
    heads of a pair issued to disjoint PE row groups (tile_position) so
    they run concurrently; AV matmuls likewise via column groups.
  - bias (bf16 from host) added into the logits PSUM bank by an
    identity-matmul before the QK matmul accumulates on top; ACT exp then
    yields the softmax denominator for free via accum_out (no
    max-subtraction: logits are bounded ~|8|).
  - 1/denominator folded into the E-transpose: transpose rhs is
    diag(1/d) (built by tensor_scalar identity*rd) instead of identity,
    so out = E^T @ diag(rd) scales each q-column at zero extra cost.
  - mask (uint8, transposed [k,q] on host) folded into the mandatory
    PSUM->SBUF copy of the transposed weights (tensor_tensor mult).
  - o-projection from avT (heads stacked) accumulated over head pairs;
    bf16 partial out.
"""

import sys

for p in ("/opt/trn_rl_repo",):
    if p not in sys.path:
        sys.path.insert(0, p)

import numpy as np
import ml_dtypes

import concourse.bass as bass
import concourse.bacc as bacc
import concourse.mybir as mybir
import concourse.tile as tile
from concourse.bass import ts, ds
from concourse.masks import make_identity

F32 = mybir.dt.float32
BF16 = mybir.dt.bfloat16
U8 = mybir.dt.uint8
AX = mybir.AxisListType
OP = mybir.AluOpType
ACTF = mybir.ActivationFunctionType

A = 1024      # d_model
C = 64        # d_k = d_v
HP = 4        # heads per core
NAT = A // 128  # 8 a-tiles

# Tunables
MASK_U8 = True     # mask as uint8 (less DMA, 1x DVE) vs bf16 (2x DVE)
DIAG_RD = False     # fold 1/denominator into transpose rhs diag(rd)


def build_program(LQ=2048, LK=2048):
    nc = bacc.Bacc(None, target_bir_lowering=False)
    NQT, NKT = LQ // 128, LK // 128
    NQC, NKC = LQ // 512, LK // 512
    MDT = U8 if MASK_U8 else BF16

    QTd = nc.declare_dram_parameter("QT", [A, LQ], BF16, isOutput=False)
    KTd = nc.declare_dram_parameter("KT", [A, LK], BF16, isOutput=False)
    VTd = nc.declare_dram_parameter("VT", [A, LK], BF16, isOutput=False)
    # bias/mask transposed to [k,q] and tiled per (head, q-chunk):
    # [h, qc, p(=k%128), kt, q] so one (h,qc) slab is a contiguous DMA.
    biasd = nc.declare_dram_parameter(
        "bias", [HP, NQC, 128, NKT, 512], BF16, isOutput=False)
    maskd = nc.declare_dram_parameter(
        "maskt", [HP, NQC, 128, NKT, 512], MDT, isOutput=False)
    qwd = nc.declare_dram_parameter("qw", [128, NAT, 2, 128], BF16, isOutput=False)
    kwd = nc.declare_dram_parameter("kw", [128, NAT, 2, 128], BF16, isOutput=False)
    vwd = nc.declare_dram_parameter("vw", [128, NAT, 2 * 128], BF16, isOutput=False)
    gwd = nc.declare_dram_parameter("gw", [128, NAT, 2, 128], BF16, isOutput=False)
    gbd = nc.declare_dram_parameter("gb", [128, 2], F32, isOutput=False)
    owd = nc.declare_dram_parameter("ow", [128, 2, A], BF16, isOutput=False)
    outd = nc.declare_dram_parameter("out", [LQ, A], BF16, isOutput=True)

    with tile.TileContext(nc) as tc:
        with (
            tc.tile_pool(name="const", bufs=1) as cp,
            tc.tile_pool(name="proj", bufs=1) as pp,
        ):
            identb = cp.tile([128, 128], BF16)
            make_identity(nc, identb)

            wq = cp.tile([128, NAT, 2, 128], BF16)
            wk = cp.tile([128, NAT, 2, 128], BF16)
            wg = cp.tile([128, NAT, 2, 128], BF16)
            wv = cp.tile([128, NAT, 2 * 128], BF16)
            for w, d in ((wq, qwd), (wk, kwd), (wg, gwd)):
                nc.sync.dma_start(out=w, in_=d[:, :, :, :])
            nc.sync.dma_start(out=wv, in_=vwd[:, :, :])
            wo = cp.tile([128, 2, A], BF16)
            nc.sync.dma_start(out=wo, in_=owd[:, :, :])
            gb = cp.tile([128, 2], F32)
            nc.sync.dma_start(out=gb, in_=gbd[:, :])

            # persistent projections (head pairs stacked on partitions)
            qT = pp.tile([128, 2, LQ], BF16)
            kT = pp.tile([128, 2, LK], BF16)
            gT = pp.tile([128, 2, LQ], BF16)
            v4 = pp.tile([128, NKT, HP * C], BF16)
            afin = pp.tile([128, 2, LQ], BF16)

            # ---------------- Phase 1: projections ----------------------
            with (
                tc.tile_pool(name="p1x", bufs=2) as p1x,
                tc.tile_pool(name="p1ps", bufs=3, space="PSUM") as p1p,
            ):
                def load_slab(xd, L):
                    xt = p1x.tile([128, NAT, L], BF16, tag="xt")
                    for i in range(NAT):
                        nc.sync.dma_start(out=xt[:, i, :], in_=xd[ts(i, 128), :])
                    return xt

                def project_pair(XT, w, dst, nlc, sigmoid=False):
                    """dst[:, hp, ch*512:...] = (w_pair^T @ XT)"""
                    for hp in range(2):
                        for ch in range(nlc):
                            pt = p1p.tile([128, 512], F32, tag="pq")
                            for i in range(NAT):
                                nc.tensor.matmul(
                                    pt,
                                    w[:, i, hp, :],
                                    XT[:, i, ts(ch, 512)],
                                    start=(i == 0),
                                    stop=(i == NAT - 1),
                                )
                            if sigmoid:
                                nc.scalar.activation(
                                    dst[:, hp, ts(ch, 512)],
                                    pt,
                                    ACTF.Sigmoid,
                                    bias=gb[:, hp : hp + 1],
                                )
                            else:
                                nc.vector.tensor_copy(dst[:, hp, ts(ch, 512)], pt)

                XTq = load_slab(QTd, LQ)
                project_pair(XTq, wq, qT, NQC)
                project_pair(XTq, wg, gT, NQC, sigmoid=True)

                XTk = load_slab(KTd, LK)
                project_pair(XTk, wk, kT, NKC)

                XTv = load_slab(VTd, LK)
                for kt in range(NKT):
                    pt = p1p.tile([128, HP * C], F32, tag="pv")
                    for i in range(NAT):
                        nc.tensor.matmul(
                            pt,
                            XTv[:, i, ts(kt, 128)],
                            wv[:, i, :],
                            start=(i == 0),
                            stop=(i == NAT - 1),
                        )
                    nc.vector.tensor_copy(v4[:, kt, :], pt)

            # ---------------- Phase 2: attention (logits in [k,q]) ------
            # Per (pair, qc): logits^T [k,q] per k-tile = kT.T@qT + bias^T
            # (identity-mm); exp -> E^T bf16 in SBUF directly (no PE
            # transposes). Denominator d[q] = ones^T @ E^T via accumulated
            # ones-matmuls; mask applied by DVE mult into smT; AV
            # accumulates over k-tiles (head pair in PE column groups).
            # rd = 1/d broadcast to [64,512] by a rank-1 ones matmul and
            # applied together with the gate at the [c,q] stage.
            with (
                tc.tile_pool(name="Ep", bufs=3) as Ep,
                tc.tile_pool(name="bp", bufs=3) as bp,
                tc.tile_pool(name="mp", bufs=3) as mp,
                tc.tile_pool(name="rp", bufs=4) as rp,
                tc.tile_pool(name="smsb", bufs=6) as smsb,
                tc.tile_pool(name="avsb", bufs=2) as avsb,
                tc.tile_pool(name="lgp", bufs=2, space="PSUM") as lgp,
                tc.tile_pool(name="dp", bufs=2, space="PSUM") as dpp,
                tc.tile_pool(name="avp", bufs=1, space="PSUM") as avp,
                tc.tile_pool(name="rbp", bufs=1, space="PSUM") as rbp,
            ):
                ones1 = cp.tile([1, C], F32)
                nc.gpsimd.memset(ones1, 1.0)
                onesc = cp.tile([128, 1], BF16)
                nc.gpsimd.memset(onesc, 1.0)

                for hp in range(2):
                    for qc in range(NQC):
                        # stream in bias^T / mask^T tiles for this (pair,qc)
                        bts, mks, Es, dts = [], [], [], []
                        for h01 in range(2):
                            h = 2 * hp + h01
                            bt = bp.tile([128, NKT, 512], BF16, tag="bt")
                            nc.sync.dma_start(out=bt, in_=biasd[h, qc])
                            bts.append(bt)
                            mk = mp.tile([128, NKT, 512], MDT, tag="mk")
                            nc.sync.dma_start(out=mk, in_=maskd[h, qc])
                            mks.append(mk)
                            Es.append(Ep.tile([128, NKT, 512], BF16,
                                               name="E", tag="E"))
                            dts.append(dpp.tile([1, 512], F32,
                                                name="dt", tag="dt"))
                        av = avp.tile([128, 512], F32, tag="av")
                        for kt2 in range(NKT // 2):
                            lgs = []
                            for h01 in range(2):
                                pb = 64 * h01
                                lg = lgp.tile([128, 2, 512], F32, tag="lg")
                                for j in range(2):
                                    kt = 2 * kt2 + j
                                    nc.tensor.matmul(
                                        lg[:, j, :],
                                        identb,
                                        bts[h01][:, kt, :],
                                        start=True,
                                        stop=False,
                                    )
                                    nc.tensor.matmul(
                                        lg[:, j, :],
                                        kT[ds(pb, 64), hp, ts(kt, 128)],
                                        qT[ds(pb, 64), hp, ts(qc, 512)],
                                        start=False,
                                        stop=True,
                                        tile_position=(pb, 0),
                                    )
                                lgs.append(lg)
                            for h01 in range(2):
                                h = 2 * hp + h01
                                pb = 64 * h01
                                nc.scalar.activation(
                                    Es[h01][:, ds(2 * kt2, 2), :],
                                    lgs[h01],
                                    ACTF.Exp,
                                )
                                for j in range(2):
                                    kt = 2 * kt2 + j
                                    # denominator: dT += ones^T @ E^T
                                    nc.tensor.matmul(
                                        dts[h01],
                                        onesc,
                                        Es[h01][:, kt, :],
                                        start=(kt == 0),
                                        stop=(kt == NKT - 1),
                                    )
                                    smT = smsb.tile([128, 512], BF16, tag="smT")
                                    nc.vector.tensor_mul(
                                        smT, Es[h01][:, kt, :], mks[h01][:, kt, :]
                                    )
                                    nc.tensor.matmul(
                                        av[ds(pb, 64), :],
                                        v4[:, kt, ts(h, C)],
                                        smT,
                                        start=(kt == 0),
                                        stop=(kt == NKT - 1),
                                        tile_position=(0, pb),
                                    )
                        # rd broadcast + gate
                        rdb = rbp.tile([128, 512], F32, tag="rdb")
                        for h01 in range(2):
                            rd = rp.tile([1, 512], F32, tag="rd")
                            nc.vector.reciprocal(rd, dts[h01])
                            nc.tensor.matmul(
                                rdb[ds(64 * h01, 64), :],
                                ones1,
                                rd,
                                start=True,
                                stop=True,
                                tile_position=(0, 64 * h01),
                            )
                        avg = avsb.tile([128, 512], BF16, tag="avg")
                        nc.vector.tensor_mul(
                            avg, av, gT[:, hp, ts(qc, 512)]
                        )
                        nc.vector.tensor_mul(
                            afin[:, hp, ts(qc, 512)], avg, rdb
                        )

            # ---------------- Phase 3: o-projection ---------------------
            with (
                tc.tile_pool(name="op", bufs=2, space="PSUM") as opp,
                tc.tile_pool(name="ob", bufs=3) as obp,
            ):
                for qt in range(NQT):
                    for oc in range(2):
                        op = opp.tile([128, 512], F32, tag="op")
                        for hp in range(2):
                            nc.tensor.matmul(
                                op,
                                afin[:, hp, ts(qt, 128)],
                                wo[:, hp, ts(oc, 512)],
                                start=(hp == 0),
                                stop=(hp == 1),
                            )
                        ob = obp.tile([128, 512], BF16, tag="ob")
                        nc.vector.tensor_copy(ob, op)
                        nc.sync.dma_start(
                            out=outd[ts(qt, 128), ts(oc, 512)], in_=ob
                        )

    nc.finalize()
    return nc


def make_in_maps(Q, K, V, bias, mask, q_weights, k_weights, v_weights,
                 g_weights, g_bias, o_weights, LQ, LK):
    """Shard full inputs into 8 per-core input maps (host does layout)."""
    bf = ml_dtypes.bfloat16
    scale = float(C) ** -0.5
    B, H = Q.shape[0], q_weights.shape[1]

    # per-batch transposed inputs, shared across the 4 cores of the batch
    QT = [np.ascontiguousarray(np.asarray(Q[b], np.float32).T.astype(bf))
          for b in range(B)]
    KT = [np.ascontiguousarray(np.asarray(K[b], np.float32).T.astype(bf))
          for b in range(B)]
    VT = [np.ascontiguousarray(np.asarray(V[b], np.float32).T.astype(bf))
          for b in range(B)]
    NQC, NKT = LQ // 512, LK // 128

    def tile_kq(arr_hkq, dt):
        # [H, k, q] -> [H, qc, p, kt, q512] (k = 128*kt + p, q = 512*qc + q512)
        H = arr_hkq.shape[0]
        v = arr_hkq.reshape(H, NKT, 128, NQC, 512)
        return np.ascontiguousarray(v.transpose(0, 3, 2, 1, 4)).astype(dt)

    # transpose to [k, q] then tile; per batch over all heads
    bias_t = [tile_kq(np.asarray(bias[b], np.float32).transpose(0, 2, 1), bf)
              for b in range(B)]
    mdt = np.uint8 if MASK_U8 else bf
    mask_t = [tile_kq(np.asarray(mask[b]).transpose(0, 2, 1), mdt)
              for b in range(B)]

    def pack_pair_w(w4):
        # [1024, 4, 64] -> [128, 8, 2, 128]
        w = np.ascontiguousarray(w4).reshape(A, 2, 128)
        return np.ascontiguousarray(
            w.reshape(NAT, 128, 2, 128).transpose(1, 0, 2, 3)).astype(bf)

    in_maps = []
    for core in range(8):
        b, h0 = (core // 4) % B, (4 * (core % 4)) % H
        gbarr = np.zeros((128, 2), np.float32)
        for h in range(HP):
            gbarr[64 * (h % 2): 64 * (h % 2) + 64, h // 2] = g_bias[h0 + h]
        # v weights natural rhs layout [128, 8, 256]
        wv4 = np.ascontiguousarray(v_weights[:, h0:h0 + HP, :]).reshape(A, 256)
        wv_packed = np.ascontiguousarray(
            wv4.reshape(NAT, 128, 256).transpose(1, 0, 2)).astype(bf)
        # o weights [128 (c-stack of h01), 2 (pair), 1024]
        ow = np.zeros((128, 2, A), np.float32)
        for hp in range(2):
            for h01 in range(2):
                ow[64 * h01:64 * h01 + 64, hp, :] = \
                    o_weights[h0 + 2 * hp + h01]
        in_maps.append({
            "QT": QT[b],
            "KT": KT[b],
            "VT": VT[b],
            "bias": bias_t[b][h0:h0 + HP],
            "maskt": mask_t[b][h0:h0 + HP],
            "qw": pack_pair_w(q_weights[:, h0:h0 + HP, :] * scale),
            "kw": pack_pair_w(k_weights[:, h0:h0 + HP, :]),
            "vw": wv_packed,
            "gw": pack_pair_w(g_weights[:, h0:h0 + HP, :]),
            "gb": gbarr,
            "ow": ow.astype(bf),
        })
    return in_maps


_NC_CACHE = {}


def kernel(Q, K, V, bias, mask, q_weights, k_weights, v_weights,
           g_weights, g_bias, o_weights, o_bias, trace=False):
    from concourse.bass_utils import run_bass_kernel_spmd

    B, LQ, _ = Q.shape
    LK = K.shape[1]
    key = (LQ, LK)
    if key not in _NC_CACHE:
        _NC_CACHE[key] = build_program(LQ, LK)
    nc = _NC_CACHE[key]

    in_maps = make_in_maps(Q, K, V, bias, mask, q_weights, k_weights,
                           v_weights, g_weights, g_bias, o_weights, LQ, LK)
    res = run_bass_kernel_spmd(nc, in_maps, core_ids=list(range(8)),
                               trace=trace)
    outs = [m["out"] for m in res.results]
    full = np.zeros((B, LQ, A), np.float32)
    for core in range(8):
        full[core // 4] += np.asarray(outs[core], np.float32)
    full += np.asarray(o_bias, np.float32)[None, None, :]
    if trace:
        kernel.last_exec_time_ns = res.exec_time_ns
    return full


# revision 20
# speedup vs baseline: 1.1907x; 1.1907x over previous
"""Trainium2 Bass kernel for gated multi-head attention (AlphaFold-style).

Reference computation (per batch b):
  q = Q @ qw * dk^-0.5; k = K @ kw; v = V @ vw           (per-head projections)
  logits = q @ k^T + bias; W = softmax(logits)
  W = where(mask, W, 0)                                   (post-softmax mask)
  av = W @ v; gate = sigmoid(Q @ gw + g_bias); av *= gate
  out = av @ o_w + o_bias

Sharding: 8 cores; core i handles batch b=i//4 and 4 heads h0=4*(i%4).
Each core returns a partial [LQ, D_MODEL] output (its heads' o-projection
contribution, bf16); host sums the partials per batch and adds o_bias.

v2 design (all-bf16 compute, PE-lean):
  - Host pre-transposes Q,K,V to [A, L] bf16 -> no on-device input
    transposes; projections read XT slabs directly as lhsT/rhs.
  - Head pairs stacked on partitions (base 0/64); QK matmuls for the two
    heads of a pair issued to disjoint PE row groups (tile_position) so
    they run concurrently; AV matmuls likewise via column groups.
  - bias (bf16 from host) added into the logits PSUM bank by an
    identity-matmul before the QK matmul accumulates on top; ACT exp then
    yields the softmax denominator for free via accum_out (no
    max-subtraction: logits are bounded ~|8|).
  - 1/denominator folded into the E-transpose: transpose rhs is
    diag(1/d) (built by tensor_scalar identity*rd) instead of identity,
    so out = E^T @ diag(rd) scales each q-column at zero extra cost.
  - mask (uint8, transposed [k,q] on host) folded into the mandatory
    PSUM->SBUF copy of the transposed weights (tensor_tensor mult).
  - o-projection from avT (heads stacked) accumulated over head pairs;
    bf16 partial out.
"""

import sys

for p in ("/opt/trn_rl_repo",):
    if p not in sys.path:
        sys.path.insert(0, p)

import numpy as np
import ml_dtypes

import concourse.bass as bass
import concourse.bacc as bacc
import concourse.mybir as mybir
import concourse.tile as tile
from concourse.bass import ts, ds
from concourse.masks import make_identity

F32 = mybir.dt.float32
F32R = mybir.dt.float32r
BF16 = mybir.dt.bfloat16
U8 = mybir.dt.uint8
AX = mybir.AxisListType
OP = mybir.AluOpType
ACTF = mybir.ActivationFunctionType

A = 1024      # d_model
C = 64        # d_k = d_v
HP = 4        # heads per core
NAT = A // 128  # 8 a-tiles

# Tunables
MASK_U8 = True     # mask as uint8 (less DMA, 1x DVE) vs bf16 (2x DVE)
DIAG_RD = False     # fold 1/denominator into transpose rhs diag(rd)


def build_program(LQ=2048, LK=2048):
    nc = bacc.Bacc(None, target_bir_lowering=False)
    NQT, NKT = LQ // 128, LK // 128
    NQC, NKC = LQ // 512, LK // 512
    MDT = U8 if MASK_U8 else BF16

    QTd = nc.declare_dram_parameter("QT", [A, LQ], BF16, isOutput=False)
    KTd = nc.declare_dram_parameter("KT", [A, LK], BF16, isOutput=False)
    VTd = nc.declare_dram_parameter("VT", [A, LK], BF16, isOutput=False)
    # bias/mask transposed to [k,q] and tiled per (head, q-chunk):
    # [h, qc, p(=k%128), kt, q] so one (h,qc) slab is a contiguous DMA.
    biasd = nc.declare_dram_parameter(
        "bias", [HP, NQC, 128, NKT, 512], BF16, isOutput=False)
    maskd = nc.declare_dram_parameter(
        "maskt", [HP, NQC, 128, NKT, 512], MDT, isOutput=False)
    qwd = nc.declare_dram_parameter("qw", [128, NAT, 2, 128], BF16, isOutput=False)
    kwd = nc.declare_dram_parameter("kw", [128, NAT, 2, 128], BF16, isOutput=False)
    vwd = nc.declare_dram_parameter("vw", [128, NAT, 2 * 128], BF16, isOutput=False)
    gwd = nc.declare_dram_parameter("gw", [128, NAT, 2, 128], BF16, isOutput=False)
    gbd = nc.declare_dram_parameter("gb", [128, 2], F32, isOutput=False)
    owd = nc.declare_dram_parameter("ow", [128, 2, A], BF16, isOutput=False)
    hseld = nc.declare_dram_parameter("hsel", [2, 128], F32R, isOutput=False)
    outd = nc.declare_dram_parameter("out", [LQ, A], BF16, isOutput=True)

    with tile.TileContext(nc) as tc:
        with (
            tc.tile_pool(name="const", bufs=1) as cp,
            tc.tile_pool(name="proj", bufs=1) as pp,
        ):
            identb = cp.tile([128, 128], BF16)
            make_identity(nc, identb)

            wq = cp.tile([128, NAT, 2, 128], BF16)
            wk = cp.tile([128, NAT, 2, 128], BF16)
            wg = cp.tile([128, NAT, 2, 128], BF16)
            wv = cp.tile([128, NAT, 2 * 128], BF16)
            for w, d in ((wq, qwd), (wk, kwd), (wg, gwd)):
                nc.sync.dma_start(out=w, in_=d[:, :, :, :])
            nc.sync.dma_start(out=wv, in_=vwd[:, :, :])
            wo = cp.tile([128, 2, A], BF16)
            nc.sync.dma_start(out=wo, in_=owd[:, :, :])
            gb = cp.tile([128, 2], F32)
            nc.sync.dma_start(out=gb, in_=gbd[:, :])

            # persistent projections (head pairs stacked on partitions)
            qT = pp.tile([128, 2, LQ], F32R)
            kT = pp.tile([128, 2, LK], F32R)
            gT = pp.tile([128, 2, LQ], BF16)
            v4 = pp.tile([128, NKT, HP * C], BF16)
            afin = pp.tile([128, 2, LQ], BF16)

            # ---------------- Phase 1: projections ----------------------
            with (
                tc.tile_pool(name="p1x", bufs=2) as p1x,
                tc.tile_pool(name="p1ps", bufs=3, space="PSUM") as p1p,
            ):
                def load_slab(xd, L):
                    xt = p1x.tile([128, NAT, L], BF16, tag="xt")
                    for i in range(NAT):
                        nc.sync.dma_start(out=xt[:, i, :], in_=xd[ts(i, 128), :])
                    return xt

                def project_pair(XT, w, dst, nlc, sigmoid=False):
                    """dst[:, hp, ch*512:...] = (w_pair^T @ XT)"""
                    for hp in range(2):
                        for ch in range(nlc):
                            pt = p1p.tile([128, 512], F32, tag="pq")
                            for i in range(NAT):
                                nc.tensor.matmul(
                                    pt,
                                    w[:, i, hp, :],
                                    XT[:, i, ts(ch, 512)],
                                    start=(i == 0),
                                    stop=(i == NAT - 1),
                                )
                            if sigmoid:
                                nc.scalar.activation(
                                    dst[:, hp, ts(ch, 512)],
                                    pt,
                                    ACTF.Sigmoid,
                                    bias=gb[:, hp : hp + 1],
                                )
                            else:
                                nc.vector.tensor_copy(dst[:, hp, ts(ch, 512)], pt)

                XTq = load_slab(QTd, LQ)
                project_pair(XTq, wq, qT, NQC)
                project_pair(XTq, wg, gT, NQC, sigmoid=True)

                XTk = load_slab(KTd, LK)
                project_pair(XTk, wk, kT, NKC)

                XTv = load_slab(VTd, LK)
                for kt in range(NKT):
                    pt = p1p.tile([128, HP * C], F32, tag="pv")
                    for i in range(NAT):
                        nc.tensor.matmul(
                            pt,
                            XTv[:, i, ts(kt, 128)],
                            wv[:, i, :],
                            start=(i == 0),
                            stop=(i == NAT - 1),
                        )
                    nc.vector.tensor_copy(v4[:, kt, :], pt)

            # ---------------- Phase 2: attention (logits in [k,q]) ------
            # Logits computed transposed [k,q]: lg(kt) = bias^T (identity-
            # matmul) + kT.T@qT (f32r full-rate, head pair on PE row
            # groups).  exp -> E^T bf16 straight into SBUF (no PE
            # transposes).  Denominator d[q] = ones^T @ E^T accumulated by
            # one-hot matmuls into a shared [2,512] PSUM tile; mask applied
            # by DVE mult into smT; AV accumulates over k-tiles (head pair
            # in PE column groups).  rd=1/d is broadcast to both head
            # row-blocks by a single rank-2 f32r matmul and applied with
            # the gate at the small [c,q] stage.
            with (
                tc.tile_pool(name="Ep", bufs=3) as Ep,
                tc.tile_pool(name="bp", bufs=3) as bp,
                tc.tile_pool(name="mp", bufs=3) as mp,
                tc.tile_pool(name="rp", bufs=2) as rp,
                tc.tile_pool(name="smsb", bufs=4) as smsb,
                tc.tile_pool(name="avsb", bufs=2) as avsb,
                tc.tile_pool(name="lgp", bufs=4, space="PSUM") as lgp,
                tc.tile_pool(name="dp", bufs=1, space="PSUM") as dpp,
                tc.tile_pool(name="avp", bufs=2, space="PSUM") as avp,
                tc.tile_pool(name="rbp", bufs=1, space="PSUM") as rbp,
            ):
                # oneh2[:, j, :]: ones in column j -> d-matmul routes head j
                # to row j of the shared [2, 512] accumulator.
                oneh2 = cp.tile([128, 2, 2], BF16)
                nc.gpsimd.memset(oneh2, 0.0)
                nc.gpsimd.memset(oneh2[:, 0, 0:1], 1.0)
                nc.gpsimd.memset(oneh2[:, 1, 1:2], 1.0)
                # hsel: row j ones over column block j -> rd broadcast
                hsel = cp.tile([2, 128], F32R)
                nc.sync.dma_start(out=hsel, in_=hseld[:, :])

                for hp in range(2):
                    for qc in range(NQC):
                        bts, mks, Es = [], [], []
                        for h01 in range(2):
                            h = 2 * hp + h01
                            bt = bp.tile([128, NKT, 512], BF16, tag="bt")
                            nc.sync.dma_start(out=bt, in_=biasd[h, qc])
                            bts.append(bt)
                            mk = mp.tile([128, NKT, 512], MDT, tag="mk")
                            nc.sync.dma_start(out=mk, in_=maskd[h, qc])
                            mks.append(mk)
                            Es.append(Ep.tile([128, NKT, 512], BF16,
                                              name="E", tag="E"))
                        dt = dpp.tile([2, 512], F32, tag="dt")
                        av = avp.tile([128, 512], F32, tag="av")
                        for kt in range(NKT):
                            lgs = []
                            for h01 in range(2):
                                lg = lgp.tile([128, 512], F32,
                                              name="lg", tag="lg")
                                nc.tensor.matmul(
                                    lg,
                                    identb,
                                    bts[h01][:, kt, :],
                                    start=True,
                                    stop=False,
                                )
                                lgs.append(lg)
                            for h01 in range(2):
                                pb = 64 * h01
                                nc.tensor.matmul(
                                    lgs[h01],
                                    kT[ds(pb, 64), hp, ts(kt, 128)],
                                    qT[ds(pb, 64), hp, ts(qc, 512)],
                                    start=False,
                                    stop=True,
                                    tile_position=(pb, 0),
                                )
                            for h01 in range(2):
                                nc.scalar.activation(
                                    Es[h01][:, kt, :],
                                    lgs[h01],
                                    ACTF.Exp,
                                )
                            for h01 in range(2):
                                # denominator rows via one-hot matmul
                                nc.tensor.matmul(
                                    dt,
                                    oneh2[:, h01, :],
                                    Es[h01][:, kt, :],
                                    start=(kt == 0 and h01 == 0),
                                    stop=(kt == NKT - 1 and h01 == 1),
                                    skip_group_check=True,
                                )
                            for h01 in range(2):
                                h = 2 * hp + h01
                                pb = 64 * h01
                                smT = smsb.tile([128, 512], BF16,
                                                name="smT", tag="smT")
                                nc.vector.tensor_mul(
                                    smT, Es[h01][:, kt, :], mks[h01][:, kt, :]
                                )
                                nc.tensor.matmul(
                                    av[ds(pb, 64), :],
                                    v4[:, kt, ts(h, C)],
                                    smT,
                                    start=(kt == 0),
                                    stop=(kt == NKT - 1),
                                    tile_position=(0, pb),
                                )
                        # rd = 1/d ; broadcast both heads by one f32r matmul
                        rd2 = rp.tile([2, 512], F32R, tag="rd2")
                        with nc.allow_low_precision(reason="f32r rd (tf32 ~2^-11 rel, fine)"):
                            nc.vector.reciprocal(rd2, dt)
                        rdb = rbp.tile([128, 512], F32, tag="rdb")
                        nc.tensor.matmul(
                            rdb,
                            hsel,
                            rd2,
                            start=True,
                            stop=True,
                        )
                        avg = avsb.tile([128, 512], BF16, tag="avg")
                        nc.vector.tensor_mul(
                            avg, av, gT[:, hp, ts(qc, 512)]
                        )
                        nc.vector.tensor_mul(
                            afin[:, hp, ts(qc, 512)], avg, rdb
                        )

            # ---------------- Phase 3: o-projection ---------------------
            with (
                tc.tile_pool(name="op", bufs=2, space="PSUM") as opp,
                tc.tile_pool(name="ob", bufs=3) as obp,
            ):
                for qt in range(NQT):
                    for oc in range(2):
                        op = opp.tile([128, 512], F32, tag="op")
                        for hp in range(2):
                            nc.tensor.matmul(
                                op,
                                afin[:, hp, ts(qt, 128)],
                                wo[:, hp, ts(oc, 512)],
                                start=(hp == 0),
                                stop=(hp == 1),
                            )
                        ob = obp.tile([128, 512], BF16, tag="ob")
                        nc.vector.tensor_copy(ob, op)
                        nc.sync.dma_start(
                            out=outd[ts(qt, 128), ts(oc, 512)], in_=ob
                        )

    nc.finalize()
    return nc


def make_in_maps(Q, K, V, bias, mask, q_weights, k_weights, v_weights,
                 g_weights, g_bias, o_weights, LQ, LK):
    """Shard full inputs into 8 per-core input maps (host does layout)."""
    bf = ml_dtypes.bfloat16
    scale = float(C) ** -0.5
    B, H = Q.shape[0], q_weights.shape[1]

    # per-batch transposed inputs, shared across the 4 cores of the batch
    QT = [np.ascontiguousarray(np.asarray(Q[b], np.float32).T.astype(bf))
          for b in range(B)]
    KT = [np.ascontiguousarray(np.asarray(K[b], np.float32).T.astype(bf))
          for b in range(B)]
    VT = [np.ascontiguousarray(np.asarray(V[b], np.float32).T.astype(bf))
          for b in range(B)]
    NQC, NKT = LQ // 512, LK // 128

    def tile_kq(arr_hkq, dt):
        # [H, k, q] -> [H, qc, p, kt, q512] (k = 128*kt + p, q = 512*qc + q512)
        H = arr_hkq.shape[0]
        v = arr_hkq.reshape(H, NKT, 128, NQC, 512)
        return np.ascontiguousarray(v.transpose(0, 3, 2, 1, 4)).astype(dt)

    # transpose to [k, q] then tile; per batch over all heads
    bias_t = [tile_kq(np.asarray(bias[b], np.float32).transpose(0, 2, 1), bf)
              for b in range(B)]
    mdt = np.uint8 if MASK_U8 else bf
    mask_t = [tile_kq(np.asarray(mask[b]).transpose(0, 2, 1), mdt)
              for b in range(B)]

    def pack_pair_w(w4):
        # [1024, 4, 64] -> [128, 8, 2, 128]
        w = np.ascontiguousarray(w4).reshape(A, 2, 128)
        return np.ascontiguousarray(
            w.reshape(NAT, 128, 2, 128).transpose(1, 0, 2, 3)).astype(bf)

    hsel_const = np.zeros((2, 128), np.float32)
    hsel_const[0, 0:64] = 1.0
    hsel_const[1, 64:128] = 1.0

    in_maps = []
    for core in range(8):
        b, h0 = (core // 4) % B, (4 * (core % 4)) % H
        gbarr = np.zeros((128, 2), np.float32)
        for h in range(HP):
            gbarr[64 * (h % 2): 64 * (h % 2) + 64, h // 2] = g_bias[h0 + h]
        # v weights natural rhs layout [128, 8, 256]
        wv4 = np.ascontiguousarray(v_weights[:, h0:h0 + HP, :]).reshape(A, 256)
        wv_packed = np.ascontiguousarray(
            wv4.reshape(NAT, 128, 256).transpose(1, 0, 2)).astype(bf)
        # o weights [128 (c-stack of h01), 2 (pair), 1024]
        ow = np.zeros((128, 2, A), np.float32)
        for hp in range(2):
            for h01 in range(2):
                ow[64 * h01:64 * h01 + 64, hp, :] = \
                    o_weights[h0 + 2 * hp + h01]
        in_maps.append({
            "QT": QT[b],
            "KT": KT[b],
            "VT": VT[b],
            "bias": bias_t[b][h0:h0 + HP],
            "maskt": mask_t[b][h0:h0 + HP],
            "qw": pack_pair_w(q_weights[:, h0:h0 + HP, :] * scale),
            "kw": pack_pair_w(k_weights[:, h0:h0 + HP, :]),
            "vw": wv_packed,
            "gw": pack_pair_w(g_weights[:, h0:h0 + HP, :]),
            "gb": gbarr,
            "ow": ow.astype(bf),
            "hsel": hsel_const,
        })
    return in_maps


_NC_CACHE = {}


def kernel(Q, K, V, bias, mask, q_weights, k_weights, v_weights,
           g_weights, g_bias, o_weights, o_bias, trace=False):
    from concourse.bass_utils import run_bass_kernel_spmd

    B, LQ, _ = Q.shape
    LK = K.shape[1]
    key = (LQ, LK)
    if key not in _NC_CACHE:
        _NC_CACHE[key] = build_program(LQ, LK)
    nc = _NC_CACHE[key]

    in_maps = make_in_maps(Q, K, V, bias, mask, q_weights, k_weights,
                           v_weights, g_weights, g_bias, o_weights, LQ, LK)
    res = run_bass_kernel_spmd(nc, in_maps, core_ids=list(range(8)),
                               trace=trace)
    outs = [m["out"] for m in res.results]
    full = np.zeros((B, LQ, A), np.float32)
    for core in range(8):
        full[core // 4] += np.asarray(outs[core], np.float32)
    full += np.asarray(o_bias, np.float32)[None, None, :]
    if trace:
        kernel.last_exec_time_ns = res.exec_time_ns
    return full
